# revision 12
# baseline (speedup 1.0000x reference)
"""Longformer block-diagonal self-attention on 8 Trainium2 NeuronCores.

Sharding: core = batch*2 + head_group.  Each core handles one batch (S=4096)
and 8 of the 16 heads (a 512-wide slice of the embedding).  Per-core output is
the partial out-projection (O_g @ Wo_g); host sums the two head-group partials
per batch and adds the constant row (bv @ Wo + bo).

Algebraic simplifications:
  - bk drops out of softmax entirely (constant over the key axis per query).
  - bv contributes bv @ Wo to every position (softmax rows sum to 1) -> host.
  - 1/sqrt(head_dim) folded into Wq/bq on host.

Attention is computed transposed (S^T = K^T-major) so the probability matrix
comes out with keys on partitions -- exactly the layout the PV matmul needs as
its moving operand.  No transposes anywhere:
  - scores:  sT[k, q] = kt^T @ qt           (K=64 contraction, row-tiled pairs)
  - softmax: exp on ACT; key-axis sums via ones-vector matmul (M=1);
             1/sum broadcast across partitions via K=1 ones outer product;
             normalization fused into the PSUM->SBUF evacuation tensor_tensor.
  - PV:      po[d, q] = v^T @ pT            (col-tiled head pairs)
"""

import numpy as np
import ml_dtypes

import concourse.bass as bass
import concourse.tile as tile
from concourse import bacc, mybir
from concourse.bass_utils import run_bass_kernel_spmd

F32 = mybir.dt.float32
BF16 = mybir.dt.bfloat16

B, S, E = 4, 4096, 1024
H, D, W = 8, 64, 256          # per-core heads, head dim, window
EG = H * D                    # 512: per-core embedding slice
CHUNK = 512
NCHUNK = S // CHUNK
NW_CHUNK = CHUNK // W         # windows per chunk
KT_E = E // 128               # contraction tiles over E
N_ETILE = EG // 128           # 4 e'-tiles per group

_NC_CACHE = {}


def _build_nc(niter=0):
    nc = bacc.Bacc("TRN2", target_bir_lowering=False, debug=False, num_devices=8)
    xt = nc.dram_tensor("xt", [E, S], BF16, kind="ExternalInput").ap()
    wq = nc.dram_tensor("wq", [E, EG], BF16, kind="ExternalInput").ap()
    wk = nc.dram_tensor("wk", [E, EG], BF16, kind="ExternalInput").ap()
    wv = nc.dram_tensor("wv", [E, EG], BF16, kind="ExternalInput").ap()
    wo = nc.dram_tensor("wo", [EG, E], BF16, kind="ExternalInput").ap()
    bq = nc.dram_tensor("bq", [EG, 1], F32, kind="ExternalInput").ap()
    out = nc.dram_tensor("out", [S, E], BF16, kind="ExternalOutput").ap()

    with tile.TileContext(nc) as tc:
        _body(tc, nc, xt, wq, wk, wv, wo, bq, out, niter)
    nc.compile()
    return nc


def _chunk_body(tc, nc, pools, wts, xt, out, c):
    (consts, xpool, qkv, attn, otp, fo,
     ps_big, ps_st, ps_po, ps_rb, ps_sm) = pools
    wq_t, wk_t, wv_t, wo_t, bq_t, ones128, ones1 = wts
    s0 = c * CHUNK

    xc = []
    for k in range(KT_E):
        t = xpool.tile([128, CHUNK], BF16, name=f"xc{k}", tag=f"xc{k}")
        nc.sync.dma_start(t[:], xt[k * 128:(k + 1) * 128, s0:s0 + CHUNK])
        xc.append(t)

    # ---- phase 1: QT, KT (e'-major), V (s-major) ----
    qt, kt = [], []
    for t in range(N_ETILE):
        pq = ps_big.tile([128, CHUNK], F32, name=f"pq{t}", tag="big")
        for k in range(KT_E):
            nc.tensor.matmul(pq[:], wq_t[k][:, t * 128:(t + 1) * 128],
                             xc[k][:], start=(k == 0), stop=(k == KT_E - 1))
        q_sb = qkv.tile([128, CHUNK], BF16, name=f"qt{t}", tag=f"qt{t}")
        nc.vector.tensor_scalar_add(q_sb[:], pq[:], bq_t[t][:])
        qt.append(q_sb)

        pk = ps_big.tile([128, CHUNK], F32, name=f"pk{t}", tag="big")
        for k in range(KT_E):
            nc.tensor.matmul(pk[:], wk_t[k][:, t * 128:(t + 1) * 128],
                             xc[k][:], start=(k == 0), stop=(k == KT_E - 1))
        k_sb = qkv.tile([128, CHUNK], BF16, name=f"kt{t}", tag=f"kt{t}")
        nc.scalar.copy(k_sb[:], pk[:])
        kt.append(k_sb)

    vt = []
    for t in range(CHUNK // 128):
        pv = ps_big.tile([128, EG], F32, name=f"pv{t}", tag="big")
        for k in range(KT_E):
            nc.tensor.matmul(pv[:], xc[k][:, t * 128:(t + 1) * 128],
                             wv_t[k][:], start=(k == 0), stop=(k == KT_E - 1))
        v_sb = qkv.tile([128, EG], BF16, name=f"vt{t}", tag=f"vt{t}")
        nc.scalar.copy(v_sb[:], pv[:])
        vt.append(v_sb)

    # ---- phase 2: windowed attention, transposed scores, no transposes ----
    ot = [otp.tile([128, CHUNK], BF16, name=f"ot{t}", tag=f"ot{t}")
          for t in range(N_ETILE)]
    for wl in range(NW_CHUNK):
        k0 = wl * W
        for et in range(N_ETILE):          # head pair (2*et, 2*et+1)
            porb = ps_po.tile([128, 2 * W], F32, name=f"porb{et}_{wl}",
                              tag="porb")
            po, rb = porb[:, 0:W], porb[:, W:2 * W]
            sums = ps_sm.tile([128, W], F32, name=f"sm{et}_{wl}", tag="sm")
            for sub in range(2):           # head within pair
                prow = sub * 64
                st = ps_st.tile([128, 2 * W], F32, name=f"st{et}{sub}{wl}",
                                tag="st")
                for kb in range(2):        # key half-window (128 keys each)
                    nc.tensor.matmul(
                        st[:, kb * W:(kb + 1) * W],
                        kt[et][prow:prow + 64,
                               k0 + kb * 128:k0 + (kb + 1) * 128],
                        qt[et][prow:prow + 64, k0:k0 + W],
                        start=(kb == 0), stop=(kb == 1))
                pT = attn.tile([128, 2 * W], BF16, name=f"pT{et}{sub}{wl}",
                               tag=f"pT{sub}")
                nc.scalar.activation(pT[:], st[:],
                                     mybir.ActivationFunctionType.Exp)
                for kb in range(2):
                    nc.tensor.matmul(sums[32 * sub:32 * sub + 1, :],
                                     ones128[:], pT[:, kb * W:(kb + 1) * W],
                                     start=(kb == 0), stop=(kb == 1))
                for kb in range(2):
                    h = 2 * et + sub
                    nc.tensor.matmul(po[prow:prow + 64, :],
                                     vt[wl * 2 + kb][:, h * 64:(h + 1) * 64],
                                     pT[:, kb * W:(kb + 1) * W],
                                     start=(kb == 0), stop=(kb == 1))
                rec = attn.tile([1, W], BF16, name=f"rec{et}{sub}{wl}",
                                tag=f"rec{sub}")
                with nc.allow_low_precision(reason="softmax scale in bf16"):
                    nc.vector.reciprocal(rec[:],
                                         sums[32 * sub:32 * sub + 1, :])
                nc.tensor.matmul(rb[prow:prow + 64, :], ones1[:], rec[:],
                                 start=True, stop=True)
            po_sb = attn.tile([128, W], BF16, name=f"posb{et}_{wl}", tag="posb")
            nc.scalar.copy(po_sb[:], po)
            nc.vector.tensor_tensor(ot[et][:, k0:k0 + W], po_sb[:], rb,
                                    op=mybir.AluOpType.mult)

    # ---- phase 3: out-projection partial ----
    for t in range(CHUNK // 128):
        f_sb = fo.tile([128, E], BF16, name=f"f{t}", tag="fout")
        for eh in range(2):
            pf = ps_big.tile([128, 512], F32, name=f"pf{t}_{eh}", tag="big")
            for k4 in range(N_ETILE):
                nc.tensor.matmul(pf[:],
                                 ot[k4][:, t * 128:(t + 1) * 128],
                                 wo_t[k4][:, eh * 512:(eh + 1) * 512],
                                 start=(k4 == 0), stop=(k4 == N_ETILE - 1))
            nc.vector.tensor_copy(f_sb[:, eh * 512:(eh + 1) * 512], pf[:])
        nc.sync.dma_start(out[s0 + t * 128:s0 + (t + 1) * 128, :], f_sb[:])


def _body(tc, nc, xt, wq, wk, wv, wo, bq, out, niter):
    from contextlib import ExitStack
    ctx = ExitStack()
    with ctx:
        consts = ctx.enter_context(tc.tile_pool(name="consts", bufs=1))
        xpool = ctx.enter_context(tc.tile_pool(name="xpool", bufs=4))
        qkv = ctx.enter_context(tc.tile_pool(name="qkv", bufs=3))
        attn = ctx.enter_context(tc.tile_pool(name="attn", bufs=6))
        otp = ctx.enter_context(tc.tile_pool(name="otp", bufs=3))
        fo = ctx.enter_context(tc.tile_pool(name="fo", bufs=4))
        ps_big = ctx.enter_context(tc.tile_pool(name="ps_big", bufs=2, space="PSUM"))
        ps_st = ctx.enter_context(tc.tile_pool(name="ps_st", bufs=2, space="PSUM"))
        ps_po = ctx.enter_context(tc.tile_pool(name="ps_po", bufs=2, space="PSUM"))
        ps_rb = None
        ps_sm = ctx.enter_context(tc.tile_pool(name="ps_sm", bufs=2, space="PSUM"))

        ones128 = consts.tile([128, 1], BF16, name="ones128")
        nc.vector.memset(ones128[:], 1.0)
        ones1 = consts.tile([1, 64], BF16, name="ones1")
        nc.vector.memset(ones1[:], 1.0)
        wq_t = [consts.tile([128, EG], BF16, name=f"wq{k}") for k in range(KT_E)]
        wk_t = [consts.tile([128, EG], BF16, name=f"wk{k}") for k in range(KT_E)]
        wv_t = [consts.tile([128, EG], BF16, name=f"wv{k}") for k in range(KT_E)]
        wo_t = [consts.tile([128, E], BF16, name=f"wo{k}") for k in range(N_ETILE)]
        bq_t = [consts.tile([128, 1], F32, name=f"bq{k}") for k in range(N_ETILE)]
        for k in range(KT_E):
            nc.gpsimd.dma_start(wq_t[k][:], wq[k * 128:(k + 1) * 128, :])
            nc.gpsimd.dma_start(wk_t[k][:], wk[k * 128:(k + 1) * 128, :])
            nc.gpsimd.dma_start(wv_t[k][:], wv[k * 128:(k + 1) * 128, :])
        for k in range(N_ETILE):
            nc.gpsimd.dma_start(wo_t[k][:], wo[k * 128:(k + 1) * 128, :])
            nc.gpsimd.dma_start(bq_t[k][:], bq[k * 128:(k + 1) * 128, :])

        pools = (consts, xpool, qkv, attn, otp, fo,
                 ps_big, ps_st, ps_po, ps_rb, ps_sm)
        wts = (wq_t, wk_t, wv_t, wo_t, bq_t, ones128, ones1)
        if niter:
            with tc.For_i(0, niter, 1) as _i:
                for c in range(NCHUNK):
                    _chunk_body(tc, nc, pools, wts, xt, out, c)
        else:
            for c in range(NCHUNK):
                _chunk_body(tc, nc, pools, wts, xt, out, c)


def _in_maps(x, Wq, bq, Wk, Wv, Wo):
    bf = ml_dtypes.bfloat16
    sc = np.float32(1.0 / np.sqrt(D))
    in_maps = []
    for core in range(8):
        b, g = core // 2, core % 2
        gs = slice(g * EG, (g + 1) * EG)
        in_maps.append({
            "xt": np.ascontiguousarray(x[b].T).astype(bf),
            "wq": (Wq[:, gs] * sc).astype(bf),
            "wk": np.ascontiguousarray(Wk[:, gs]).astype(bf),
            "wv": np.ascontiguousarray(Wv[:, gs]).astype(bf),
            "wo": np.ascontiguousarray(Wo[gs, :]).astype(bf),
            "bq": (bq[gs] * sc).astype(np.float32).reshape(EG, 1),
        })
    return in_maps


def kernel(x, Wq, bq, Wk, bk, Wv, bv, Wo, bo):
    x, Wq, bq = np.asarray(x), np.asarray(Wq), np.asarray(bq)
    Wk, Wv, Wo = np.asarray(Wk), np.asarray(Wv), np.asarray(Wo)
    bv, bo = np.asarray(bv), np.asarray(bo)

    if "nc" not in _NC_CACHE:
        _NC_CACHE["nc"] = _build_nc()
    nc = _NC_CACHE["nc"]

    res = run_bass_kernel_spmd(nc, _in_maps(x, Wq, bq, Wk, Wv, Wo),
                               core_ids=list(range(8)))
    const_row = (bv.astype(np.float64) @ Wo.astype(np.float64)
                 + bo.astype(np.float64)).astype(np.float32)
    out = np.empty((B, S, E), np.float32)
    for b in range(B):
        out[b] = (res.results[2 * b]["out"].astype(np.float32)
                  + res.results[2 * b + 1]["out"].astype(np.float32)
                  + const_row)
    return out


# revision 13
# speedup vs baseline: 1.2128x; 1.2128x over previous
"""Longformer block-diagonal self-attention on 8 Trainium2 NeuronCores.

Sharding: core = batch*2 + head_group.  Each core handles one batch (S=4096)
and 8 of the 16 heads (a 512-wide slice of the embedding).  Per-core output is
the partial out-projection (O_g @ Wo_g); host sums the two head-group partials
per batch and adds the constant row (bv @ Wo + bo).

Algebraic simplifications:
  - bk drops out of softmax entirely (constant over the key axis per query).
  - bv contributes bv @ Wo to every position (softmax rows sum to 1) -> host.
  - 1/sqrt(head_dim) folded into Wq/bq on host.

Attention is computed transposed (S^T = K^T-major) so the probability matrix
comes out with keys on partitions -- exactly the layout the PV matmul needs as
its moving operand.  No transposes anywhere:
  - scores:  sT[k, q] = kt^T @ qt           (K=64 contraction, row-tiled pairs)
  - softmax: exp on ACT; key-axis sums via ones-vector matmul (M=1);
             1/sum broadcast across partitions via K=1 ones outer product;
             normalization fused into the PSUM->SBUF evacuation tensor_tensor.
  - PV:      po[d, q] = v^T @ pT            (col-tiled head pairs)
"""

import numpy as np
import ml_dtypes

import concourse.bass as bass
import concourse.tile as tile
from concourse import bacc, mybir
from concourse.bass_utils import run_bass_kernel_spmd

F32 = mybir.dt.float32
BF16 = mybir.dt.bfloat16

B, S, E = 4, 4096, 1024
H, D, W = 8, 64, 256          # per-core heads, head dim, window
EG = H * D                    # 512: per-core embedding slice
CHUNK = 512
NCHUNK = S // CHUNK
NW_CHUNK = CHUNK // W         # windows per chunk
KT_E = E // 128               # contraction tiles over E
N_ETILE = EG // 128           # 4 e'-tiles per group

_NC_CACHE = {}


def _build_nc(niter=0):
    nc = bacc.Bacc("TRN2", target_bir_lowering=False, debug=False, num_devices=8)
    xt = nc.dram_tensor("xt", [E, S], BF16, kind="ExternalInput").ap()
    wq = nc.dram_tensor("wq", [E, EG], BF16, kind="ExternalInput").ap()
    wk = nc.dram_tensor("wk", [E, EG], BF16, kind="ExternalInput").ap()
    wv = nc.dram_tensor("wv", [E, EG], BF16, kind="ExternalInput").ap()
    wo = nc.dram_tensor("wo", [EG, E], BF16, kind="ExternalInput").ap()
    bq = nc.dram_tensor("bq", [EG, 1], F32, kind="ExternalInput").ap()
    out = nc.dram_tensor("out", [S, E], BF16, kind="ExternalOutput").ap()

    with tile.TileContext(nc) as tc:
        _body(tc, nc, xt, wq, wk, wv, wo, bq, out, niter)
    nc.compile()
    return nc


def _chunk_body(tc, nc, pools, wts, xt, out, c):
    (consts, xpool, qkv, attn, otp, fo,
     ps_big, ps_st, ps_po, ps_rb, ps_sm) = pools
    wq_t, wk_t, wv_t, wo_t, bq_t, ones128, ones1 = wts
    s0 = c * CHUNK

    xc = []
    for k in range(KT_E):
        t = xpool.tile([128, CHUNK], BF16, name=f"xc{k}", tag=f"xc{k}")
        nc.sync.dma_start(t[:], xt[k * 128:(k + 1) * 128, s0:s0 + CHUNK])
        xc.append(t)

    # ---- phase 1: QT, KT (e'-major), V (s-major) ----
    qt, kt = [], []
    for t in range(N_ETILE):
        pq = ps_big.tile([128, CHUNK], F32, name=f"pq{t}", tag="big")
        for k in range(KT_E):
            nc.tensor.matmul(pq[:], wq_t[k][:, t * 128:(t + 1) * 128],
                             xc[k][:], start=(k == 0), stop=(k == KT_E - 1))
        q_sb = qkv.tile([128, CHUNK], BF16, name=f"qt{t}", tag=f"qt{t}")
        nc.vector.tensor_scalar_add(q_sb[:], pq[:], bq_t[t][:])
        qt.append(q_sb)

        pk = ps_big.tile([128, CHUNK], F32, name=f"pk{t}", tag="big")
        for k in range(KT_E):
            nc.tensor.matmul(pk[:], wk_t[k][:, t * 128:(t + 1) * 128],
                             xc[k][:], start=(k == 0), stop=(k == KT_E - 1))
        k_sb = qkv.tile([128, CHUNK], BF16, name=f"kt{t}", tag=f"kt{t}")
        nc.scalar.copy(k_sb[:], pk[:])
        kt.append(k_sb)

    vt = []
    for t in range(CHUNK // 128):
        pv = ps_big.tile([128, EG], F32, name=f"pv{t}", tag="big")
        for k in range(KT_E):
            nc.tensor.matmul(pv[:], xc[k][:, t * 128:(t + 1) * 128],
                             wv_t[k][:], start=(k == 0), stop=(k == KT_E - 1))
        v_sb = qkv.tile([128, EG], BF16, name=f"vt{t}", tag=f"vt{t}")
        nc.scalar.copy(v_sb[:], pv[:])
        vt.append(v_sb)

    # ---- phase 2: windowed attention, transposed scores, no transposes ----
    ot = [otp.tile([128, CHUNK], BF16, name=f"ot{t}", tag=f"ot{t}")
          for t in range(N_ETILE)]
    for wl in range(NW_CHUNK):
        k0 = wl * W
        for et in range(N_ETILE):          # head pair (2*et, 2*et+1)
            porb = ps_po.tile([128, 2 * W], F32, name=f"porb{et}_{wl}",
                              tag="porb")
            po, rb = porb[:, 0:W], porb[:, W:2 * W]
            sums = ps_sm.tile([128, W], F32, name=f"sm{et}_{wl}", tag="sm")
            for sub in range(2):           # head within pair
                prow = sub * 64
                st = ps_st.tile([128, 2 * W], F32, name=f"st{et}{sub}{wl}",
                                tag="st")
                for kb in range(2):        # key half-window (128 keys each)
                    nc.tensor.matmul(
                        st[:, kb * W:(kb + 1) * W],
                        kt[et][prow:prow + 64,
                               k0 + kb * 128:k0 + (kb + 1) * 128],
                        qt[et][prow:prow + 64, k0:k0 + W],
                        start=(kb == 0), stop=(kb == 1))
                pT = attn.tile([128, 2 * W], BF16, name=f"pT{et}{sub}{wl}",
                               tag=f"pT{sub}")
                nc.scalar.activation(pT[:], st[:],
                                     mybir.ActivationFunctionType.Exp)
                for kb in range(2):
                    nc.tensor.matmul(sums[32 * sub:32 * sub + 1, :],
                                     ones128[:], pT[:, kb * W:(kb + 1) * W],
                                     start=(kb == 0), stop=(kb == 1))
                for kb in range(2):
                    h = 2 * et + sub
                    nc.tensor.matmul(po[prow:prow + 64, :],
                                     vt[wl * 2 + kb][:, h * 64:(h + 1) * 64],
                                     pT[:, kb * W:(kb + 1) * W],
                                     start=(kb == 0), stop=(kb == 1))
                rec = attn.tile([1, W], BF16, name=f"rec{et}{sub}{wl}",
                                tag=f"rec{sub}")
                with nc.allow_low_precision(reason="softmax scale in bf16"):
                    nc.vector.reciprocal(rec[:],
                                         sums[32 * sub:32 * sub + 1, :])
                nc.tensor.matmul(rb[prow:prow + 64, :], ones1[:], rec[:],
                                 start=True, stop=True)
            po_sb = attn.tile([128, W], BF16, name=f"posb{et}_{wl}", tag="posb")
            nc.scalar.copy(po_sb[:], po)
            nc.vector.tensor_tensor(ot[et][:, k0:k0 + W], po_sb[:], rb,
                                    op=mybir.AluOpType.mult)

    # ---- phase 3: out-projection partial ----
    for t in range(CHUNK // 128):
        f_sb = fo.tile([128, E], BF16, name=f"f{t}", tag="fout")
        for eh in range(2):
            pf = ps_big.tile([128, 512], F32, name=f"pf{t}_{eh}", tag="big")
            for k4 in range(N_ETILE):
                nc.tensor.matmul(pf[:],
                                 ot[k4][:, t * 128:(t + 1) * 128],
                                 wo_t[k4][:, eh * 512:(eh + 1) * 512],
                                 start=(k4 == 0), stop=(k4 == N_ETILE - 1))
            nc.vector.tensor_copy(f_sb[:, eh * 512:(eh + 1) * 512], pf[:])
        nc.sync.dma_start(out[s0 + t * 128:s0 + (t + 1) * 128, :], f_sb[:])


def _body(tc, nc, xt, wq, wk, wv, wo, bq, out, niter):
    from contextlib import ExitStack
    ctx = ExitStack()
    with ctx:
        consts = ctx.enter_context(tc.tile_pool(name="consts", bufs=1))
        xpool = ctx.enter_context(tc.tile_pool(name="xpool", bufs=3))
        qkv = ctx.enter_context(tc.tile_pool(name="qkv", bufs=2))
        attn = ctx.enter_context(tc.tile_pool(name="attn", bufs=4))
        otp = ctx.enter_context(tc.tile_pool(name="otp", bufs=2))
        fo = ctx.enter_context(tc.tile_pool(name="fo", bufs=3))
        ps_big = ctx.enter_context(tc.tile_pool(name="ps_big", bufs=2, space="PSUM"))
        ps_st = ctx.enter_context(tc.tile_pool(name="ps_st", bufs=2, space="PSUM"))
        ps_po = ctx.enter_context(tc.tile_pool(name="ps_po", bufs=2, space="PSUM"))
        ps_rb = None
        ps_sm = ctx.enter_context(tc.tile_pool(name="ps_sm", bufs=2, space="PSUM"))

        ones128 = consts.tile([128, 1], BF16, name="ones128")
        nc.vector.memset(ones128[:], 1.0)
        ones1 = consts.tile([1, 64], BF16, name="ones1")
        nc.vector.memset(ones1[:], 1.0)
        wq_t = [consts.tile([128, EG], BF16, name=f"wq{k}") for k in range(KT_E)]
        wk_t = [consts.tile([128, EG], BF16, name=f"wk{k}") for k in range(KT_E)]
        wv_t = [consts.tile([128, EG], BF16, name=f"wv{k}") for k in range(KT_E)]
        wo_t = [consts.tile([128, E], BF16, name=f"wo{k}") for k in range(N_ETILE)]
        bq_t = [consts.tile([128, 1], F32, name=f"bq{k}") for k in range(N_ETILE)]
        for k in range(KT_E):
            nc.gpsimd.dma_start(wq_t[k][:], wq[k * 128:(k + 1) * 128, :])
            nc.gpsimd.dma_start(wk_t[k][:], wk[k * 128:(k + 1) * 128, :])
            nc.gpsimd.dma_start(wv_t[k][:], wv[k * 128:(k + 1) * 128, :])
        for k in range(N_ETILE):
            nc.gpsimd.dma_start(wo_t[k][:], wo[k * 128:(k + 1) * 128, :])
            nc.gpsimd.dma_start(bq_t[k][:], bq[k * 128:(k + 1) * 128, :])

        pools = (consts, xpool, qkv, attn, otp, fo,
                 ps_big, ps_st, ps_po, ps_rb, ps_sm)
        wts = (wq_t, wk_t, wv_t, wo_t, bq_t, ones128, ones1)
        if niter:
            with tc.For_i(0, niter, 1) as _i:
                for c in range(NCHUNK):
                    _chunk_body(tc, nc, pools, wts, xt, out, c)
        else:
            for c in range(NCHUNK):
                _chunk_body(tc, nc, pools, wts, xt, out, c)


def _in_maps(x, Wq, bq, Wk, Wv, Wo):
    bf = ml_dtypes.bfloat16
    sc = np.float32(1.0 / np.sqrt(D))
    in_maps = []
    for core in range(8):
        b, g = core // 2, core % 2
        gs = slice(g * EG, (g + 1) * EG)
        in_maps.append({
            "xt": np.ascontiguousarray(x[b].T).astype(bf),
            "wq": (Wq[:, gs] * sc).astype(bf),
            "wk": np.ascontiguousarray(Wk[:, gs]).astype(bf),
            "wv": np.ascontiguousarray(Wv[:, gs]).astype(bf),
            "wo": np.ascontiguousarray(Wo[gs, :]).astype(bf),
            "bq": (bq[gs] * sc).astype(np.float32).reshape(EG, 1),
        })
    return in_maps


def kernel(x, Wq, bq, Wk, bk, Wv, bv, Wo, bo):
    x, Wq, bq = np.asarray(x), np.asarray(Wq), np.asarray(bq)
    Wk, Wv, Wo = np.asarray(Wk), np.asarray(Wv), np.asarray(Wo)
    bv, bo = np.asarray(bv), np.asarray(bo)

    if "nc" not in _NC_CACHE:
        _NC_CACHE["nc"] = _build_nc()
    nc = _NC_CACHE["nc"]

    res = run_bass_kernel_spmd(nc, _in_maps(x, Wq, bq, Wk, Wv, Wo),
                               core_ids=list(range(8)))
    const_row = (bv.astype(np.float64) @ Wo.astype(np.float64)
                 + bo.astype(np.float64)).astype(np.float32)
    out = np.empty((B, S, E), np.float32)
    for b in range(B):
        out[b] = (res.results[2 * b]["out"].astype(np.float32)
                  + res.results[2 * b + 1]["out"].astype(np.float32)
                  + const_row)
    return out
